# revision 38
# baseline (speedup 1.0000x reference)
"""Trainium2 Bass kernel for EnhancedTripletLoss (hard-mining triplet loss).

Strategy (8 NeuronCores, SPMD, no collectives) — v4:
  * Anchors sharded BY CLASS (8 classes == 8 cores), capped at Mc=1024 rows
    (Mt=8 tiles).  The few anchors beyond 1024 in oversized classes (~40 of
    8192 for this input) are folded in on the host in float64 — every core
    runs a uniform 8-tile program.
  * Columns are permuted into 8 class blocks PADDED to exactly 1024 columns
    (duplicate columns are min-neutral); excess columns of large classes
    live in a shared OVERFLOW region padded to uniform 16-wide per-class
    sub-blocks (one multi-dim reduce covers all of them).  Own-class
    columns are sign-negated so min-reductions yield both
    hardest-positive (max) and hardest-negative (min) stats.
  * ONE fp8 DoubleRow matmul per 512-column segment computes the whole
    biased Gram slab: k-tile0 = embedding dims 0..127, k-tile1 = dims
    128..253 plus TWO ||e_j||^2 rows (2 fp8 terms of sq/2 against a
    stationary 2.0), so psum = -2<a,e>_254 + sq_j directly — no separate
    sq channel, no accumulation chains.  Each matmul is a complete
    start/stop group writing its own PSUM bank; with rotating banks the
    PE sustains 1 column/cycle at K=256 (2x the bf16 MAC rate; measured
    216 ns per segment).  Dropping dims 254/255 from the cross term adds
    zero-mean noise comparable to the fp8 quantization itself; host-sim
    total loss error vs the fp32 reference: ~3.5e-3 (tolerance 2e-2).
  * Reduce: per block the Scalar engine converts the second 512-column
    half PSUM->SBUF fp16; a custom DVE ucode op MIN2_RED then computes
    accum = min(init, min_k min(psum_half0[k], sbuf_half1[k])) in one
    pass — the DVE's two read ports (one PSUM + one SBUF) eat the block
    at 2 columns/cycle.  MIN2_RED's elementwise output is discarded
    in-place into the already-consumed PSUM half (a stride-0 SBUF
    broadcast out stalls the DVE ~15%).
  * All per-anchor-tile epilogues batch into ONE tail pass: block minima
    accumulate into a resident [128, Mt*NB] tile; mask-add + multi-dim
    min-reduce + relu/sqrt/margin run once over all 8 anchor tiles.
    Each core writes per-partition partial sums [128, 2]; the host
    combines (plus the float64 terms of the host-folded extra anchors).
"""

import numpy as np
import ml_dtypes

P = 128          # SBUF partitions
D = 256          # embedding dim (fixed by the problem)
ND = 254         # dims kept in the cross term (2 rows fold the sq channel)
NCLS = 8         # number of classes == number of cores
NCORES = 8
MARGIN = 0.3
BIGM = 1.0e30    # block-exclusion additive mask (applied to bmins stats only)
WMAIN = 1024     # main block width (padded): 2 fp32 PSUM banks
MC = 1024        # anchor rows per core (excess anchors -> host)
OVPAD = 16       # overflow sub-block width (uniform, padded)
WARMUP = 4       # PE clock-ramp matmuls during the DMA fill

F8 = ml_dtypes.float8_e4m3

_OPS = None


def _get_ops():
    """Define + register the custom DVE ucode ops (once per process)."""
    global _OPS
    if _OPS is not None:
        return _OPS
    import concourse.dve_ops as dve_ops
    import concourse.dve_spec as ds
    from concourse.dve_uop import AluOp

    def _ref_add(in0, in1, c0, c1, c2):
        b = in0.astype(np.float32) + in1
        return b, np.minimum(
            c0, b.reshape(b.shape[0], -1).min(axis=-1, keepdims=True))

    def _ref_min2(in0, in1, c0, c1, c2):
        b = np.minimum(in0.astype(np.float32), in1)
        return b, np.minimum(
            c0, b.reshape(b.shape[0], -1).min(axis=-1, keepdims=True))

    defs = [
        ("ADD_MIN_RED", ds.Src0 + ds.Src1, _ref_add,
         {"v3": "3b1a86e7a42a7109", "v4": "c551ceffaec94a3a"}),
        ("MIN2_RED", ds.minn(ds.Src0, ds.Src1), _ref_min2,
         {"v3": "80668f319ac378ba", "v4": "23f6c1536de15f6a"}),
    ]
    ops = {}
    dirty = False
    for name, body, ref, sha in defs:
        spec = ds.Spec(body=body, accum=AluOp.MIN, accum_init=ds.C0,
                       reference=ref)
        op = dve_ops.DveOp(name, spec, subdim=False, uops_sha=sha)
        if name not in dve_ops._SUB_OPCODE_FOR_NAME:
            dve_ops.OPS.append(op)
            dve_ops._SUB_OPCODE_FOR_NAME[name] = (
                dve_ops._CUSTOM_DVE_ROW_BASE + len(dve_ops.OPS) - 1)
            dve_ops.CUSTOM_DVE_SPECS[name] = spec
            dirty = True
        ops[name] = op
    if dirty:
        import concourse.bass_utils as bu
        if hasattr(bu, "_table_cache"):
            bu._table_cache.clear()
    _OPS = ops
    return ops


def _layout(counts):
    """Overflow classes (beyond the padded 1024 main block) from counts."""
    ov = [(c, int(n) - WMAIN) for c, n in enumerate(counts) if n > WMAIN]
    for cls, w in ov:
        assert w <= OVPAD, f"overflow class {cls} width {w} > {OVPAD}"
    assert len(ov) * OVPAD <= 512, "overflow region too wide"
    return tuple(ov)


def _build_program(ov):
    import concourse.tile as tile
    from concourse import bacc, mybir

    f32 = mybir.dt.float32
    f16 = mybir.dt.float16
    bf16 = mybir.dt.bfloat16
    fp8 = mybir.dt.float8e4
    OP = mybir.AluOpType
    AX = mybir.AxisListType.X
    DR = mybir.MatmulPerfMode.DoubleRow
    RELU = mybir.ActivationFunctionType.Relu
    ops = _get_ops()
    add_min, min2 = ops["ADD_MIN_RED"], ops["MIN2_RED"]

    Mt = MC // P
    nov = len(ov)
    ovw = nov * OVPAD
    NB = NCLS + nov
    NSEG = WMAIN // 512

    nc = bacc.Bacc("TRN2", target_bir_lowering=False, debug=False)

    # stationary mt-major, moving seg-major: matmul slices stay contiguous
    u8d = nc.dram_tensor("u8", [P, Mt, 2, P], fp8, kind="ExternalInput")
    v8d = [nc.dram_tensor(f"v8b{b}", [P, NSEG, 2, 512], fp8,
                          kind="ExternalInput") for b in range(NCLS)]
    if ovw:
        v8od = nc.dram_tensor("v8ov", [P, 2, ovw], fp8, kind="ExternalInput")
    sqa = nc.dram_tensor("sqa", [P, Mt], f32, kind="ExternalInput")
    vld = nc.dram_tensor("valid", [P, Mt], f32, kind="ExternalInput")
    # masks replicated Mt times: [P, Mt*NB] for the batched tail epilogue
    pbig = nc.dram_tensor("posbig", [P, Mt * NB], f32, kind="ExternalInput")
    nbig = nc.dram_tensor("negbig", [P, Mt * NB], f32, kind="ExternalInput")
    out = nc.dram_tensor("out", [P, 2], f32, kind="ExternalOutput")

    with tile.TileContext(nc) as tc:
        with (
            tc.tile_pool(name="resident", bufs=1) as res,
            tc.tile_pool(name="psum", bufs=3, space="PSUM") as pp,
            tc.tile_pool(name="povf", bufs=2, space="PSUM") as po,
            tc.tile_pool(name="c16", bufs=3) as c16,
            tc.tile_pool(name="epi", bufs=12) as epi,
        ):
            # ---- PE warmup ------------------------------------------------
            wsrc = res.tile([P, 512], bf16, tag="wsrc")
            nc.vector.memset(wsrc[:], 0.0)
            wp = pp.tile([P, WMAIN], f32, tag="pblk", name="warm")
            for _ in range(WARMUP):
                nc.tensor.matmul(wp[:, 0:512], wsrc[:, 0:P], wsrc[:, :],
                                 start=True, stop=True)

            # ---- resident loads -------------------------------------------
            # two DMA queues (sync + gpsimd); v8 blocks alternate so block b
            # lands before the pipeline consumes it.  The Scalar engine
            # issues NO DMA: its queue must reach the ACT table load + first
            # PSUM->fp16 copy as early as possible (it paces the reduce).
            u8t = res.tile([P, Mt, 2, P], fp8, tag="u8")
            nc.sync.dma_start(out=u8t[:], in_=u8d[:, :, :, :])

            qs = [nc.gpsimd, nc.sync]
            v8ts = []
            for b in range(NCLS):
                tv = res.tile([P, NSEG, 2, 512], fp8, tag=f"v8b{b}",
                              name=f"v8b{b}")
                qs[b % 2].dma_start(out=tv[:], in_=v8d[b][:, :, :, :])
                v8ts.append(tv)

            if ovw:
                ov8 = res.tile([P, 2, ovw], fp8, tag="v8ov")
                nc.gpsimd.dma_start(out=ov8[:], in_=v8od[:, :, :])
            sqat = res.tile([P, Mt], f32, tag="sqa")
            nc.gpsimd.dma_start(out=sqat[:], in_=sqa[:, :])
            vldt = res.tile([P, Mt], f32, tag="valid")
            nc.gpsimd.dma_start(out=vldt[:], in_=vld[:, :])
            pbigt = res.tile([P, Mt * NB], f32, tag="posbig")
            nc.sync.dma_start(out=pbigt[:], in_=pbig[:, :])
            nbigt = res.tile([P, Mt * NB], f32, tag="negbig")
            nc.sync.dma_start(out=nbigt[:], in_=nbig[:, :])
            bmall = res.tile([P, Mt * NB], f32, tag="bmall")

            num_sb = res.tile([P, Mt], f32, tag="num")
            pdists = res.tile([P, Mt], f32, tag="pdists")
            ndists = res.tile([P, Mt], f32, tag="ndists")
            out_sb = res.tile([P, 2], f32, tag="out")

            # ---- main loop ------------------------------------------------
            for mt in range(Mt):
                bmo = mt * NB
                for b in range(NCLS):
                    ptile = pp.tile([P, WMAIN], f32, tag="pblk", name="pblk")
                    for si in range(NSEG):
                        cs = slice(si * 512, (si + 1) * 512)
                        nc.tensor.matmul(
                            ptile[:, cs], u8t[:, mt, :, :],
                            v8ts[b][:, si, :, :],
                            start=True, stop=True, perf_mode=DR,
                        )
                    sb = c16.tile([P, 512], f16, tag="sb16", name="sb16")
                    nc.scalar.copy(sb[:, :], ptile[:, 512:1024])
                    nc.vector._custom_dve(
                        min2,
                        out=ptile[:, 0:512],
                        in0=ptile[:, 0:512], in1=sb[:, :],
                        s0=BIGM, accum_out=bmall[:, bmo + b:bmo + b + 1],
                    )

                if ovw:
                    otile = po.tile([P, ovw], f32, tag="ovf", name="ovf")
                    nc.tensor.matmul(
                        otile[:, :], u8t[:, mt, :, :], ov8[:, :, :],
                        start=True, stop=True, perf_mode=DR,
                    )
                    nc.vector.tensor_reduce(
                        bmall[:, bmo + NCLS:bmo + NCLS + nov],
                        otile[:].rearrange("p (n w) -> p n w", n=nov),
                        axis=AX, op=OP.min,
                    )

            # ---- batched epilogue (one tail pass over all mt) -------------
            tadd = epi.tile([P, Mt * NB], f32, tag="tadd")
            nc.vector.tensor_tensor(tadd[:], bmall[:], pbigt[:], op=OP.add)
            mowns = epi.tile([P, Mt], f32, tag="mowns")
            nc.vector.tensor_reduce(
                mowns[:], tadd[:].rearrange("p (m n) -> p m n", m=Mt),
                axis=AX, op=OP.min)
            nc.vector.tensor_tensor(tadd[:], bmall[:], nbigt[:], op=OP.add)
            mnegs = epi.tile([P, Mt], f32, tag="mnegs")
            nc.vector.tensor_reduce(
                mnegs[:], tadd[:].rearrange("p (m n) -> p m n", m=Mt),
                axis=AX, op=OP.min)
            # pos_d2 = relu(sqa - mowns), neg_d2 = relu(sqa + mnegs); sqrt
            pd2 = epi.tile([P, Mt], f32, tag="pd2")
            nc.vector.tensor_tensor(pd2[:], sqat[:], mowns[:],
                                    op=OP.subtract)
            nc.vector.tensor_scalar_max(pd2[:], pd2[:], 0.0)
            nc.scalar.sqrt(pdists[:], pd2[:])
            nd2 = epi.tile([P, Mt], f32, tag="nd2")
            nc.vector.tensor_tensor(nd2[:], sqat[:], mnegs[:], op=OP.add)
            nc.vector.tensor_scalar_max(nd2[:], nd2[:], 0.0)
            nc.scalar.sqrt(ndists[:], nd2[:])

            per = epi.tile([P, Mt], f32, tag="per")
            nc.vector.scalar_tensor_tensor(
                per[:], in0=pdists[:], scalar=MARGIN, in1=ndists[:],
                op0=OP.add, op1=OP.subtract,
            )
            perr = epi.tile([P, Mt], f32, tag="perr")
            nc.vector.tensor_scalar_max(perr[:], per[:], 0.0)
            nc.vector.tensor_tensor(num_sb[:], perr[:], vldt[:], op=OP.mult)

            nc.vector.tensor_reduce(out_sb[:, 0:1], num_sb[:], axis=AX,
                                    op=OP.add)
            nc.vector.tensor_reduce(out_sb[:, 1:2], vldt[:], axis=AX,
                                    op=OP.add)
            nc.sync.dma_start(out=out[:, :], in_=out_sb[:])

    nc.compile()
    return nc


def _prepare_inputs(emb, lab):
    """Host-side shard/layout prep.  Returns (in_maps, meta)."""
    B = emb.shape[0]
    assert emb.shape[1] == D
    counts = np.bincount(lab, minlength=NCLS).astype(int)
    assert counts.sum() == B

    order = np.argsort(lab, kind="stable")
    cstart = np.concatenate([[0], np.cumsum(counts)]).astype(int)

    ov = _layout(counts)
    nov = len(ov)
    ovw = nov * OVPAD
    NB = NCLS + nov
    Mt = MC // P
    NSEG = WMAIN // 512
    N = NCLS * WMAIN + ovw

    sq = np.einsum("ij,ij->i", emb, emb, dtype=np.float32)  # ||e||^2, fp32

    # column index: 8 padded main blocks, then the padded overflow region.
    # Padding duplicates a column of the same class — min-neutral.
    colidx = np.empty(N, dtype=np.int64)
    own_ranges = {c: [] for c in range(NCLS)}
    off = 0
    for c in range(NCLS):
        idx = order[cstart[c]:cstart[c + 1]][:WMAIN]
        if len(idx) == 0:
            idx = order[0:1]
        pad = np.full(WMAIN - len(idx), idx[0], dtype=np.int64)
        colidx[off:off + WMAIN] = np.concatenate([idx, pad])
        own_ranges[c].append((off, WMAIN))
        off += WMAIN
    for cls, w in ov:
        idx = order[cstart[cls] + WMAIN:cstart[cls + 1]]
        assert len(idx) == w
        pad = np.full(OVPAD - w, idx[0], dtype=np.int64)
        colidx[off:off + OVPAD] = np.concatenate([idx, pad])
        own_ranges[cls].append((off, OVPAD))
        off += OVPAD

    # anchors handled on the host: rows beyond MC in oversized classes
    extra = [order[cstart[c] + MC:cstart[c + 1]] for c in range(NCLS)]
    extra_anchors = (np.concatenate(extra) if any(len(e) for e in extra)
                     else np.empty(0, dtype=np.int64))

    # global fp8 moving rows: [256] = [dims 0..127 | dims 128..253 | sq rows]
    Eg = emb[colidx]                                    # [N, 256]
    sqc = (sq[colidx].astype(np.float64) / 2.0)
    m1 = sqc.astype(F8)                                 # fp8 hi of sq/2
    m2 = (sqc - m1.astype(np.float64)).astype(F8)       # fp8 lo

    in_maps = []
    for c in range(NCLS):
        aidx = order[cstart[c]:cstart[c + 1]][:MC]
        if len(aidx) == 0:
            aidx = order[0:1]
        npad = MC - len(aidx)
        pad = np.full(npad, aidx[0], dtype=np.int64)
        aidx_p = np.concatenate([aidx, pad])

        real = np.zeros(MC, dtype=np.float32)
        real[: len(aidx)] = 1.0
        cls_valid = 1.0 if (2 <= counts[c] <= B - 1) else 0.0
        valid = (real * cls_valid).reshape(Mt, P).T.copy()  # [128, Mt]

        sqa_t = sq[aidx_p].reshape(Mt, P).T.copy()          # [128, Mt]

        s = np.ones(N, dtype=np.float32)
        for o, w in own_ranges[c]:
            s[o:o + w] = -1.0

        posbig1 = np.zeros(NB, dtype=np.float32)
        negbig1 = np.zeros(NB, dtype=np.float32)
        bm_cls = list(range(NCLS)) + [cls for cls, _ in ov]
        for j, bc in enumerate(bm_cls):
            if bc == c:
                negbig1[j] = BIGM
            else:
                posbig1[j] = BIGM
        posbig = np.broadcast_to(np.tile(posbig1, Mt), (P, Mt * NB))
        negbig = np.broadcast_to(np.tile(negbig1, Mt), (P, Mt * NB))
        posbig = np.ascontiguousarray(posbig)
        negbig = np.ascontiguousarray(negbig)

        # fp8 moving rows [256, N]: e dims 0..253 then the two sq rows,
        # all sign-flipped (fp8 sign flips are exact)
        rows = np.empty((D, N), dtype=F8)
        rows[0:ND] = (Eg[:, 0:ND] * s[:, None]).astype(np.float32).T.astype(F8)
        rows[ND] = (m1.astype(np.float64) * s).astype(F8)
        rows[ND + 1] = (m2.astype(np.float64) * s).astype(F8)
        v8f = rows.reshape(2, P, N).transpose(1, 0, 2)      # [P, 2, N]

        # stationary rows [256, MC]: -2*anchor dims 0..253, then 2.0, 2.0
        urows = np.empty((D, MC), dtype=F8)
        urows[0:ND] = (-2.0 * emb[aidx_p][:, 0:ND]).astype(
            np.float32).T.astype(F8)
        urows[ND] = F8(2.0)
        urows[ND + 1] = F8(2.0)
        u8f = urows.reshape(2, P, MC).transpose(1, 0, 2)    # [P, 2, MC]
        u8 = np.ascontiguousarray(
            u8f.reshape(P, 2, Mt, P).transpose(0, 2, 1, 3))

        im = {
            "u8": u8,
            "sqa": sqa_t,
            "valid": valid,
            "posbig": posbig,
            "negbig": negbig,
        }
        for b in range(NCLS):
            o = b * WMAIN
            blk = v8f[:, :, o:o + WMAIN]                    # [P, 2, 1024]
            im[f"v8b{b}"] = np.ascontiguousarray(
                blk.reshape(P, 2, NSEG, 512).transpose(0, 2, 1, 3))
        if ovw:
            im["v8ov"] = np.ascontiguousarray(v8f[:, :, NCLS * WMAIN:])
        in_maps.append(im)

    meta = dict(ov=ov, Mt=Mt, N=N, extra_anchors=extra_anchors)
    return in_maps, meta


def _host_extra(emb, lab, extra_anchors):
    """Exact per-anchor terms for the few anchors not on any core."""
    if len(extra_anchors) == 0:
        return 0.0, 0.0
    e = emb.astype(np.float64)
    sq = np.einsum("ij,ij->i", e, e)
    num = 0.0
    den = 0.0
    for a in extra_anchors:
        d2 = sq + sq[a] - 2.0 * (e @ e[a])
        d = np.sqrt(np.maximum(d2, 0.0))
        same = lab == lab[a]
        same_excl = same.copy()
        same_excl[a] = False
        diff = ~same
        if not same_excl.any() or not diff.any():
            continue
        pd = d[same_excl].max()
        nd = d[diff].min()
        num += max(pd - nd + MARGIN, 0.0)
        den += 1.0
    return num, den


_PROGRAM_CACHE = {}


def _get_program(ov):
    if ov not in _PROGRAM_CACHE:
        _PROGRAM_CACHE[ov] = _build_program(ov)
    return _PROGRAM_CACHE[ov]


def _combine(results, num0, den0):
    num = float(num0)
    den = float(den0)
    for r in results:
        o = np.asarray(r["out"], dtype=np.float64)
        num += o[:, 0].sum()
        den += o[:, 1].sum()
    return np.float32(num / max(den, 1.0))


def _setup_trace_hook():
    """Register the axon NTFF profile hook if the image lacks antenv.axon_hooks."""
    import sys
    import types
    try:
        from antenv.axon_hooks import get_axon_ntff_profile_hook  # noqa: F401
        return
    except ImportError:
        pass
    import antenv
    from trn_agent_boot.trn_boot import _ntff_profile_via_ctypes

    mod = types.ModuleType("antenv.axon_hooks")
    state = {"h": None}
    mod.set_axon_ntff_profile_hook = lambda h: state.__setitem__("h", h)
    mod.get_axon_ntff_profile_hook = lambda: state["h"]
    sys.modules["antenv.axon_hooks"] = mod
    antenv.axon_hooks = mod
    mod.set_axon_ntff_profile_hook(
        _ntff_profile_via_ctypes("/opt/axon/libaxon_pjrt.so")
    )


def kernel(embeddings, labels, _trace=False):
    emb = np.ascontiguousarray(np.asarray(embeddings, dtype=np.float32))
    lab = np.asarray(labels).astype(np.int64).ravel()

    in_maps, meta = _prepare_inputs(emb, lab)
    nc = _get_program(meta["ov"])
    num0, den0 = _host_extra(emb, lab, meta["extra_anchors"])

    from concourse.bass_utils import run_bass_kernel_spmd

    if _trace:
        _setup_trace_hook()
        import concourse.bass_utils as _bu
        _bu.upload_artifacts = lambda tmpdir: tmpdir  # skip remote upload

    res = run_bass_kernel_spmd(
        nc, in_maps, core_ids=list(range(NCORES)), trace=bool(_trace),
    )
    loss = _combine(res.results, num0, den0)
    if _trace:
        return loss, res
    return loss


# revision 40
# speedup vs baseline: 1.0943x; 1.0943x over previous
"""Trainium2 Bass kernel for EnhancedTripletLoss (hard-mining triplet loss).

Strategy (8 NeuronCores, SPMD, no collectives) — v4:
  * Anchors sharded BY CLASS (8 classes == 8 cores), capped at Mc=1024 rows
    (Mt=8 tiles).  The few anchors beyond 1024 in oversized classes (~40 of
    8192 for this input) are folded in on the host in float64 — every core
    runs a uniform 8-tile program.
  * Columns are permuted into 8 class blocks PADDED to exactly 1024 columns
    (duplicate columns are min-neutral); excess columns of large classes
    live in a shared OVERFLOW region padded to uniform 16-wide per-class
    sub-blocks (one multi-dim reduce covers all of them).  Own-class
    columns are sign-negated so min-reductions yield both
    hardest-positive (max) and hardest-negative (min) stats.
  * ONE fp8 DoubleRow matmul per 512-column segment computes the whole
    biased Gram slab: k-tile0 = embedding dims 0..127, k-tile1 = dims
    128..253 plus TWO ||e_j||^2 rows (2 fp8 terms of sq/2 against a
    stationary 2.0), so psum = -2<a,e>_254 + sq_j directly — no separate
    sq channel, no accumulation chains.  Each matmul is a complete
    start/stop group writing its own PSUM bank; with rotating banks the
    PE sustains 1 column/cycle at K=256 (2x the bf16 MAC rate; measured
    216 ns per segment).  Dropping dims 254/255 from the cross term adds
    zero-mean noise comparable to the fp8 quantization itself; host-sim
    total loss error vs the fp32 reference: ~3.5e-3 (tolerance 2e-2).
  * Reduce: per block the Scalar engine converts the second 512-column
    half PSUM->SBUF fp16; a custom DVE ucode op MIN2_RED then computes
    accum = min(init, min_k min(psum_half0[k], sbuf_half1[k])) in one
    pass — the DVE's two read ports (one PSUM + one SBUF) eat the block
    at 2 columns/cycle.  MIN2_RED's elementwise output is discarded
    in-place into the already-consumed PSUM half (a stride-0 SBUF
    broadcast out stalls the DVE ~15%).
  * All per-anchor-tile epilogues batch into ONE tail pass: block minima
    accumulate into a resident [128, Mt*NB] tile; mask-add + multi-dim
    min-reduce + relu/sqrt/margin run once over all 8 anchor tiles.
    Each core writes per-partition partial sums [128, 2]; the host
    combines (plus the float64 terms of the host-folded extra anchors).
"""

import numpy as np
import ml_dtypes

P = 128          # SBUF partitions
D = 256          # embedding dim (fixed by the problem)
ND = 254         # dims kept in the cross term (2 rows fold the sq channel)
NCLS = 8         # number of classes == number of cores
NCORES = 8
MARGIN = 0.3
BIGM = 1.0e30    # block-exclusion additive mask (applied to bmins stats only)
WMAIN = 1024     # main block width (padded): 2 fp32 PSUM banks
MC = 1024        # anchor rows per core (excess anchors -> host)
OVPAD = 16       # overflow sub-block width (uniform, padded)
WARMUP = 4       # PE clock-ramp matmuls during the DMA fill

F8 = ml_dtypes.float8_e4m3

_OPS = None


def _get_ops():
    """Define + register the custom DVE ucode ops (once per process)."""
    global _OPS
    if _OPS is not None:
        return _OPS
    import concourse.dve_ops as dve_ops
    import concourse.dve_spec as ds
    from concourse.dve_uop import AluOp

    def _ref_add(in0, in1, c0, c1, c2):
        b = in0.astype(np.float32) + in1
        return b, np.minimum(
            c0, b.reshape(b.shape[0], -1).min(axis=-1, keepdims=True))

    def _ref_min2(in0, in1, c0, c1, c2):
        b = np.minimum(in0.astype(np.float32), in1)
        return b, np.minimum(
            c0, b.reshape(b.shape[0], -1).min(axis=-1, keepdims=True))

    defs = [
        ("ADD_MIN_RED", ds.Src0 + ds.Src1, _ref_add,
         {"v3": "3b1a86e7a42a7109", "v4": "c551ceffaec94a3a"}),
        ("MIN2_RED", ds.minn(ds.Src0, ds.Src1), _ref_min2,
         {"v3": "80668f319ac378ba", "v4": "23f6c1536de15f6a"}),
    ]
    ops = {}
    dirty = False
    for name, body, ref, sha in defs:
        spec = ds.Spec(body=body, accum=AluOp.MIN, accum_init=ds.C0,
                       reference=ref)
        op = dve_ops.DveOp(name, spec, subdim=False, uops_sha=sha)
        if name not in dve_ops._SUB_OPCODE_FOR_NAME:
            dve_ops.OPS.append(op)
            dve_ops._SUB_OPCODE_FOR_NAME[name] = (
                dve_ops._CUSTOM_DVE_ROW_BASE + len(dve_ops.OPS) - 1)
            dve_ops.CUSTOM_DVE_SPECS[name] = spec
            dirty = True
        ops[name] = op
    if dirty:
        import concourse.bass_utils as bu
        if hasattr(bu, "_table_cache"):
            bu._table_cache.clear()
    _OPS = ops
    return ops


def _layout(counts):
    """Overflow classes (beyond the padded 1024 main block) from counts."""
    ov = [(c, int(n) - WMAIN) for c, n in enumerate(counts) if n > WMAIN]
    for cls, w in ov:
        assert w <= OVPAD, f"overflow class {cls} width {w} > {OVPAD}"
    assert len(ov) * OVPAD <= 512, "overflow region too wide"
    return tuple(ov)


def _build_program(ov):
    import concourse.tile as tile
    from concourse import bacc, mybir

    f32 = mybir.dt.float32
    f16 = mybir.dt.float16
    bf16 = mybir.dt.bfloat16
    fp8 = mybir.dt.float8e4
    OP = mybir.AluOpType
    AX = mybir.AxisListType.X
    DR = mybir.MatmulPerfMode.DoubleRow
    RELU = mybir.ActivationFunctionType.Relu
    ops = _get_ops()
    add_min, min2 = ops["ADD_MIN_RED"], ops["MIN2_RED"]

    Mt = MC // P
    nov = len(ov)
    ovw = nov * OVPAD
    NB = NCLS + nov
    NSEG = WMAIN // 512

    nc = bacc.Bacc("TRN2", target_bir_lowering=False, debug=False)

    # stationary mt-major, moving seg-major: matmul slices stay contiguous
    u8d = nc.dram_tensor("u8", [P, Mt, 2, P], fp8, kind="ExternalInput")
    v8d = [nc.dram_tensor(f"v8b{b}", [P, NSEG, 2, 512], fp8,
                          kind="ExternalInput") for b in range(NCLS)]
    if ovw:
        v8od = nc.dram_tensor("v8ov", [P, 2, ovw], fp8, kind="ExternalInput")
    sqa = nc.dram_tensor("sqa", [P, Mt], f32, kind="ExternalInput")
    vld = nc.dram_tensor("valid", [P, Mt], f32, kind="ExternalInput")
    # masks replicated Mt times: [P, Mt*NB] for the batched tail epilogue
    pbig = nc.dram_tensor("posbig", [P, Mt * NB], f32, kind="ExternalInput")
    nbig = nc.dram_tensor("negbig", [P, Mt * NB], f32, kind="ExternalInput")
    out = nc.dram_tensor("out", [P, 2], f32, kind="ExternalOutput")

    with tile.TileContext(nc) as tc:
        with (
            tc.tile_pool(name="resident", bufs=1) as res,
            tc.tile_pool(name="psum", bufs=4, space="PSUM") as pp,
            tc.tile_pool(name="c16", bufs=4) as c16,
            tc.tile_pool(name="epi", bufs=12) as epi,
        ):
            # ---- PE warmup ------------------------------------------------
            wsrc = res.tile([P, 512], bf16, tag="wsrc")
            nc.vector.memset(wsrc[:], 0.0)
            wp = pp.tile([P, WMAIN], f32, tag="pblk", name="warm")
            for _ in range(WARMUP):
                nc.tensor.matmul(wp[:, 0:512], wsrc[:, 0:P], wsrc[:, :],
                                 start=True, stop=True)

            # ---- resident loads -------------------------------------------
            # two DMA queues (sync + gpsimd); v8 blocks alternate so block b
            # lands before the pipeline consumes it.  The Scalar engine
            # issues NO DMA: its queue must reach the ACT table load + first
            # PSUM->fp16 copy as early as possible (it paces the reduce).
            u8t = res.tile([P, Mt, 2, P], fp8, tag="u8")
            nc.sync.dma_start(out=u8t[:], in_=u8d[:, :, :, :])

            qs = [nc.gpsimd, nc.sync]
            v8ts = []
            for b in range(NCLS):
                tv = res.tile([P, NSEG, 2, 512], fp8, tag=f"v8b{b}",
                              name=f"v8b{b}")
                qs[b % 2].dma_start(out=tv[:], in_=v8d[b][:, :, :, :])
                v8ts.append(tv)

            if ovw:
                ov8 = res.tile([P, 2, ovw], fp8, tag="v8ov")
                nc.gpsimd.dma_start(out=ov8[:], in_=v8od[:, :, :])
            sqat = res.tile([P, Mt], f32, tag="sqa")
            nc.gpsimd.dma_start(out=sqat[:], in_=sqa[:, :])
            vldt = res.tile([P, Mt], f32, tag="valid")
            nc.gpsimd.dma_start(out=vldt[:], in_=vld[:, :])
            pbigt = res.tile([P, Mt * NB], f32, tag="posbig")
            nc.sync.dma_start(out=pbigt[:], in_=pbig[:, :])
            nbigt = res.tile([P, Mt * NB], f32, tag="negbig")
            nc.sync.dma_start(out=nbigt[:], in_=nbig[:, :])
            bmall = res.tile([P, Mt * NB], f32, tag="bmall")

            num_sb = res.tile([P, Mt], f32, tag="num")
            pdists = res.tile([P, Mt], f32, tag="pdists")
            ndists = res.tile([P, Mt], f32, tag="ndists")
            out_sb = res.tile([P, 2], f32, tag="out")

            # ---- main loop ------------------------------------------------
            for mt in range(Mt):
                bmo = mt * NB
                for b in range(NCLS):
                    ptile = pp.tile([P, WMAIN], f32, tag="pblk", name="pblk")
                    for si in range(NSEG):
                        cs = slice(si * 512, (si + 1) * 512)
                        nc.tensor.matmul(
                            ptile[:, cs], u8t[:, mt, :, :],
                            v8ts[b][:, si, :, :],
                            start=True, stop=True, perf_mode=DR,
                        )
                    sb = c16.tile([P, 512], f16, tag="sb16", name="sb16")
                    nc.scalar.copy(sb[:, :], ptile[:, 512:1024])
                    nc.vector._custom_dve(
                        min2,
                        out=ptile[:, 0:512],
                        in0=ptile[:, 0:512], in1=sb[:, :],
                        s0=BIGM, accum_out=bmall[:, bmo + b:bmo + b + 1],
                    )

                if ovw:
                    # overflow borrows a main-pool psum tile (uses 64 cols)
                    otile = pp.tile([P, WMAIN], f32, tag="pblk", name="ovf")
                    nc.tensor.matmul(
                        otile[:, 0:ovw], u8t[:, mt, :, :], ov8[:, :, :],
                        start=True, stop=True, perf_mode=DR,
                    )
                    nc.vector.tensor_reduce(
                        bmall[:, bmo + NCLS:bmo + NCLS + nov],
                        otile[:, 0:ovw].rearrange("p (n w) -> p n w", n=nov),
                        axis=AX, op=OP.min,
                    )

            # ---- batched epilogue (one tail pass over all mt) -------------
            tadd = epi.tile([P, Mt * NB], f32, tag="tadd")
            nc.vector.tensor_tensor(tadd[:], bmall[:], pbigt[:], op=OP.add)
            mowns = epi.tile([P, Mt], f32, tag="mowns")
            nc.vector.tensor_reduce(
                mowns[:], tadd[:].rearrange("p (m n) -> p m n", m=Mt),
                axis=AX, op=OP.min)
            nc.vector.tensor_tensor(tadd[:], bmall[:], nbigt[:], op=OP.add)
            mnegs = epi.tile([P, Mt], f32, tag="mnegs")
            nc.vector.tensor_reduce(
                mnegs[:], tadd[:].rearrange("p (m n) -> p m n", m=Mt),
                axis=AX, op=OP.min)
            # pos_d2 = relu(sqa - mowns), neg_d2 = relu(sqa + mnegs); sqrt
            pd2 = epi.tile([P, Mt], f32, tag="pd2")
            nc.vector.tensor_tensor(pd2[:], sqat[:], mowns[:],
                                    op=OP.subtract)
            nc.vector.tensor_scalar_max(pd2[:], pd2[:], 0.0)
            nc.scalar.sqrt(pdists[:], pd2[:])
            nd2 = epi.tile([P, Mt], f32, tag="nd2")
            nc.vector.tensor_tensor(nd2[:], sqat[:], mnegs[:], op=OP.add)
            nc.vector.tensor_scalar_max(nd2[:], nd2[:], 0.0)
            nc.scalar.sqrt(ndists[:], nd2[:])

            per = epi.tile([P, Mt], f32, tag="per")
            nc.vector.scalar_tensor_tensor(
                per[:], in0=pdists[:], scalar=MARGIN, in1=ndists[:],
                op0=OP.add, op1=OP.subtract,
            )
            perr = epi.tile([P, Mt], f32, tag="perr")
            nc.vector.tensor_scalar_max(perr[:], per[:], 0.0)
            nc.vector.tensor_tensor(num_sb[:], perr[:], vldt[:], op=OP.mult)

            nc.vector.tensor_reduce(out_sb[:, 0:1], num_sb[:], axis=AX,
                                    op=OP.add)
            nc.vector.tensor_reduce(out_sb[:, 1:2], vldt[:], axis=AX,
                                    op=OP.add)
            nc.sync.dma_start(out=out[:, :], in_=out_sb[:])

    nc.compile()
    return nc


def _prepare_inputs(emb, lab):
    """Host-side shard/layout prep.  Returns (in_maps, meta)."""
    B = emb.shape[0]
    assert emb.shape[1] == D
    counts = np.bincount(lab, minlength=NCLS).astype(int)
    assert counts.sum() == B

    order = np.argsort(lab, kind="stable")
    cstart = np.concatenate([[0], np.cumsum(counts)]).astype(int)

    ov = _layout(counts)
    nov = len(ov)
    ovw = nov * OVPAD
    NB = NCLS + nov
    Mt = MC // P
    NSEG = WMAIN // 512
    N = NCLS * WMAIN + ovw

    sq = np.einsum("ij,ij->i", emb, emb, dtype=np.float32)  # ||e||^2, fp32

    # column index: 8 padded main blocks, then the padded overflow region.
    # Padding duplicates a column of the same class — min-neutral.
    colidx = np.empty(N, dtype=np.int64)
    own_ranges = {c: [] for c in range(NCLS)}
    off = 0
    for c in range(NCLS):
        idx = order[cstart[c]:cstart[c + 1]][:WMAIN]
        if len(idx) == 0:
            idx = order[0:1]
        pad = np.full(WMAIN - len(idx), idx[0], dtype=np.int64)
        colidx[off:off + WMAIN] = np.concatenate([idx, pad])
        own_ranges[c].append((off, WMAIN))
        off += WMAIN
    for cls, w in ov:
        idx = order[cstart[cls] + WMAIN:cstart[cls + 1]]
        assert len(idx) == w
        pad = np.full(OVPAD - w, idx[0], dtype=np.int64)
        colidx[off:off + OVPAD] = np.concatenate([idx, pad])
        own_ranges[cls].append((off, OVPAD))
        off += OVPAD

    # anchors handled on the host: rows beyond MC in oversized classes
    extra = [order[cstart[c] + MC:cstart[c + 1]] for c in range(NCLS)]
    extra_anchors = (np.concatenate(extra) if any(len(e) for e in extra)
                     else np.empty(0, dtype=np.int64))

    # global fp8 moving rows: [256] = [dims 0..127 | dims 128..253 | sq rows]
    Eg = emb[colidx]                                    # [N, 256]
    sqc = (sq[colidx].astype(np.float64) / 2.0)
    m1 = sqc.astype(F8)                                 # fp8 hi of sq/2
    m2 = (sqc - m1.astype(np.float64)).astype(F8)       # fp8 lo

    in_maps = []
    for c in range(NCLS):
        aidx = order[cstart[c]:cstart[c + 1]][:MC]
        if len(aidx) == 0:
            aidx = order[0:1]
        npad = MC - len(aidx)
        pad = np.full(npad, aidx[0], dtype=np.int64)
        aidx_p = np.concatenate([aidx, pad])

        real = np.zeros(MC, dtype=np.float32)
        real[: len(aidx)] = 1.0
        cls_valid = 1.0 if (2 <= counts[c] <= B - 1) else 0.0
        valid = (real * cls_valid).reshape(Mt, P).T.copy()  # [128, Mt]

        sqa_t = sq[aidx_p].reshape(Mt, P).T.copy()          # [128, Mt]

        s = np.ones(N, dtype=np.float32)
        for o, w in own_ranges[c]:
            s[o:o + w] = -1.0

        posbig1 = np.zeros(NB, dtype=np.float32)
        negbig1 = np.zeros(NB, dtype=np.float32)
        bm_cls = list(range(NCLS)) + [cls for cls, _ in ov]
        for j, bc in enumerate(bm_cls):
            if bc == c:
                negbig1[j] = BIGM
            else:
                posbig1[j] = BIGM
        posbig = np.broadcast_to(np.tile(posbig1, Mt), (P, Mt * NB))
        negbig = np.broadcast_to(np.tile(negbig1, Mt), (P, Mt * NB))
        posbig = np.ascontiguousarray(posbig)
        negbig = np.ascontiguousarray(negbig)

        # fp8 moving rows [256, N]: e dims 0..253 then the two sq rows,
        # all sign-flipped (fp8 sign flips are exact)
        rows = np.empty((D, N), dtype=F8)
        rows[0:ND] = (Eg[:, 0:ND] * s[:, None]).astype(np.float32).T.astype(F8)
        rows[ND] = (m1.astype(np.float64) * s).astype(F8)
        rows[ND + 1] = (m2.astype(np.float64) * s).astype(F8)
        v8f = rows.reshape(2, P, N).transpose(1, 0, 2)      # [P, 2, N]

        # stationary rows [256, MC]: -2*anchor dims 0..253, then 2.0, 2.0
        urows = np.empty((D, MC), dtype=F8)
        urows[0:ND] = (-2.0 * emb[aidx_p][:, 0:ND]).astype(
            np.float32).T.astype(F8)
        urows[ND] = F8(2.0)
        urows[ND + 1] = F8(2.0)
        u8f = urows.reshape(2, P, MC).transpose(1, 0, 2)    # [P, 2, MC]
        u8 = np.ascontiguousarray(
            u8f.reshape(P, 2, Mt, P).transpose(0, 2, 1, 3))

        im = {
            "u8": u8,
            "sqa": sqa_t,
            "valid": valid,
            "posbig": posbig,
            "negbig": negbig,
        }
        for b in range(NCLS):
            o = b * WMAIN
            blk = v8f[:, :, o:o + WMAIN]                    # [P, 2, 1024]
            im[f"v8b{b}"] = np.ascontiguousarray(
                blk.reshape(P, 2, NSEG, 512).transpose(0, 2, 1, 3))
        if ovw:
            im["v8ov"] = np.ascontiguousarray(v8f[:, :, NCLS * WMAIN:])
        in_maps.append(im)

    meta = dict(ov=ov, Mt=Mt, N=N, extra_anchors=extra_anchors)
    return in_maps, meta


def _host_extra(emb, lab, extra_anchors):
    """Exact per-anchor terms for the few anchors not on any core."""
    if len(extra_anchors) == 0:
        return 0.0, 0.0
    e = emb.astype(np.float64)
    sq = np.einsum("ij,ij->i", e, e)
    num = 0.0
    den = 0.0
    for a in extra_anchors:
        d2 = sq + sq[a] - 2.0 * (e @ e[a])
        d = np.sqrt(np.maximum(d2, 0.0))
        same = lab == lab[a]
        same_excl = same.copy()
        same_excl[a] = False
        diff = ~same
        if not same_excl.any() or not diff.any():
            continue
        pd = d[same_excl].max()
        nd = d[diff].min()
        num += max(pd - nd + MARGIN, 0.0)
        den += 1.0
    return num, den


_PROGRAM_CACHE = {}


def _get_program(ov):
    if ov not in _PROGRAM_CACHE:
        _PROGRAM_CACHE[ov] = _build_program(ov)
    return _PROGRAM_CACHE[ov]


def _combine(results, num0, den0):
    num = float(num0)
    den = float(den0)
    for r in results:
        o = np.asarray(r["out"], dtype=np.float64)
        num += o[:, 0].sum()
        den += o[:, 1].sum()
    return np.float32(num / max(den, 1.0))


def _setup_trace_hook():
    """Register the axon NTFF profile hook if the image lacks antenv.axon_hooks."""
    import sys
    import types
    try:
        from antenv.axon_hooks import get_axon_ntff_profile_hook  # noqa: F401
        return
    except ImportError:
        pass
    import antenv
    from trn_agent_boot.trn_boot import _ntff_profile_via_ctypes

    mod = types.ModuleType("antenv.axon_hooks")
    state = {"h": None}
    mod.set_axon_ntff_profile_hook = lambda h: state.__setitem__("h", h)
    mod.get_axon_ntff_profile_hook = lambda: state["h"]
    sys.modules["antenv.axon_hooks"] = mod
    antenv.axon_hooks = mod
    mod.set_axon_ntff_profile_hook(
        _ntff_profile_via_ctypes("/opt/axon/libaxon_pjrt.so")
    )


def kernel(embeddings, labels, _trace=False):
    emb = np.ascontiguousarray(np.asarray(embeddings, dtype=np.float32))
    lab = np.asarray(labels).astype(np.int64).ravel()

    in_maps, meta = _prepare_inputs(emb, lab)
    nc = _get_program(meta["ov"])
    num0, den0 = _host_extra(emb, lab, meta["extra_anchors"])

    from concourse.bass_utils import run_bass_kernel_spmd

    if _trace:
        _setup_trace_hook()
        import concourse.bass_utils as _bu
        _bu.upload_artifacts = lambda tmpdir: tmpdir  # skip remote upload

    res = run_bass_kernel_spmd(
        nc, in_maps, core_ids=list(range(NCORES)), trace=bool(_trace),
    )
    loss = _combine(res.results, num0, den0)
    if _trace:
        return loss, res
    return loss
